# revision 17
# baseline (speedup 1.0000x reference)
"""Trainium2 Bass kernel for nn_DirectedGNNLayer (bipartite GATv2 x2).

Strategy (8 NeuronCores, SPMD — one program, per-core data):
  * Per encoder, partition TARGET (dst) nodes across the 8 cores
    (round-robin by degree rank) so each core owns the full segment
    softmax + aggregation for its nodes — no cross-core reductions.
  * Node-major layout: each supertile holds a block of nodes, NB nodes
    per partition row, each padded to the block's max degree W.
  * bf16 datapath: xl/xr tables, gathers, and all big elementwise ops
    are bf16 (DVE 2x mode needs all-bf16 packed-last-dim operands).
  * One batched indirect DMA per supertile (offset ap [128, NW]) — the
    994ns/instruction SWDGE overhead amortizes over the whole tile.
  * Engine balance: Pool does the two step-0-last-dim broadcasts
    (ee mult, msg mult); Act does prelu+exp; DVE does the packed adds
    and tree reductions (segment sums as log2-depth in-place adds).
  * Softmax max-subtraction is dropped (logits bounded ~|10| on this
    data); exp/den/alpha stay fp32 at the logits level.

kernel(**inputs) takes the FULL problem inputs and returns the FULL
(s_out, t_out) tuple, matching reference.reference().
"""
import sys
import os
import numpy as np
import ml_dtypes

sys.path.insert(0, '/opt/trn_rl_repo')

BF = ml_dtypes.bfloat16

N = 100000
D = 128
E = 800000
H = 4
C = 32
HC = H * C
NEG = 0.2
P = 128
NCORES = 8
CAP = 20      # max NB*W slots per partition row of a supertile
NBMAX = 8
CH = 8        # dense-phase tiles per chunk
NP_PAD = ((N + P - 1) // P) * P


def _patch_walrus():
    from concourse import bass_utils
    if getattr(bass_utils, "_ant_dge_patched", False):
        return
    orig = bass_utils.get_walrus_args

    def patched(*a, **k):
        return orig(*a, **k) + [
            "--dge-levels=io,scalar_dynamic_offset,vector_dynamic_offsets"
        ]

    bass_utils.get_walrus_args = patched
    bass_utils._ant_dge_patched = True


def _encoder_prep(n_nodes, x_src, x_dst, src, dst, edge_w, n_cores):
    """Geometry + per-core host arrays for one encoder.

    src/dst: int arrays [E]; segments (softmax) are over dst.
    Returns a dict; all per-core arrays have identical shapes across cores.
    """
    ne = len(dst)
    deg = np.bincount(dst, minlength=n_nodes)
    order = np.argsort(-deg, kind='stable')
    order = order[deg[order] > 0]
    K = len(order)

    core_of = np.full(n_nodes, -1, np.int32)
    pos_of = np.full(n_nodes, -1, np.int64)
    idx = np.arange(K)
    core_of[order] = (idx % n_cores).astype(np.int32)
    pos_of[order] = idx // n_cores
    n_loc = (K + n_cores - 1) // n_cores

    deg_rank = deg[order[0::n_cores]]

    Ws, NBs, starts = [], [], []
    pos = 0
    while pos < n_loc:
        W = int(deg_rank[pos]) if pos < len(deg_rank) else 1
        W = max(W, 1)
        NB = max(1, min(NBMAX, CAP // W))
        starts.append(pos)
        Ws.append(W)
        NBs.append(NB)
        pos += P * NB
    n_loc_pad = pos
    S = len(Ws)
    colO = np.zeros(S + 1, np.int64)
    for t in range(S):
        colO[t + 1] = colO[t] + NBs[t] * Ws[t]
    G = int(colO[-1])

    row_of = np.empty(n_loc_pad, np.int64)
    colb_of = np.empty(n_loc_pad, np.int64)
    for t in range(S):
        r = np.arange(P * NBs[t])
        sl = slice(starts[t], starts[t] + P * NBs[t])
        row_of[sl] = r // NBs[t]
        colb_of[sl] = colO[t] + (r % NBs[t]) * Ws[t]

    sidx = np.argsort(dst, kind='stable')
    sdst = dst[sidx]
    first = np.r_[True, sdst[1:] != sdst[:-1]]
    run_starts_pos = np.flatnonzero(first)
    run_id = np.cumsum(first) - 1
    w_sorted = np.arange(ne) - run_starts_pos[run_id]
    w_of = np.empty(ne, np.int64)
    w_of[sidx] = w_sorted

    c_e = core_of[dst]
    j_e = pos_of[dst]
    row_e = row_of[j_e]
    col_e = colb_of[j_e] + w_of

    gidx = np.zeros((n_cores, P, G), np.int32)
    ew = np.zeros((n_cores, P, G), BF)
    mask = np.full((n_cores, P, G), -1e30, BF)
    gidx[c_e, row_e, col_e] = src.astype(np.int32)
    ew[c_e, row_e, col_e] = edge_w.astype(BF)
    mask[c_e, row_e, col_e] = 0.0

    # host-packed local dst features, transposed for matmul lhsT (bf16)
    xdT = np.zeros((n_cores, D, n_loc_pad), BF)
    node_lists = []
    xdstT = np.ascontiguousarray(x_dst.T).astype(BF)
    for c in range(n_cores):
        nl = order[c::n_cores]
        node_lists.append(nl)
        xdT[c, :, :len(nl)] = xdstT[:, nl]

    # Group supertiles so each group's distinct sources stay < 32768 on
    # EVERY core (dma_gather idx is int16).  Each group gets its own
    # sub-table slice of xsT; gidx values are remapped group-locally.
    GCAP = 31500
    xsrcT = np.ascontiguousarray(x_src.T).astype(BF)
    seen = np.zeros((n_cores, n_nodes), bool)
    gcount = np.zeros(n_cores, np.int64)
    group_of = np.zeros(S, np.int64)
    groups = [[]]
    for t in range(S):
        sl = slice(int(colO[t]), int(colO[t + 1]))
        news = [np.unique(gidx[c][:, sl]) for c in range(n_cores)]
        adds = [np.count_nonzero(~seen[c][news[c]]) for c in range(n_cores)]
        if any(gcount[c] + adds[c] > GCAP for c in range(n_cores)):
            groups.append([])
            seen[:] = False
            gcount[:] = 0
        g = len(groups) - 1
        groups[g].append(t)
        group_of[t] = g
        for c in range(n_cores):
            gcount[c] += np.count_nonzero(~seen[c][news[c]])
            seen[c][news[c]] = True

    # per-group sub-tables, padded to a common (max-over-core) size
    NG = len(groups)
    grows = np.zeros(NG, np.int64)
    rows_cg = [[None] * NG for _ in range(n_cores)]
    for g in range(NG):
        lo = int(colO[groups[g][0]])
        hi = int(colO[groups[g][-1] + 1])
        for c in range(n_cores):
            u = np.unique(gidx[c][:, lo:hi])
            rows_cg[c][g] = u
            grows[g] = max(grows[g], len(u))
    grows = ((grows + P - 1) // P) * P
    goff = np.zeros(NG + 1, np.int64)
    for g in range(NG):
        goff[g + 1] = goff[g] + grows[g]
    TOT = int(goff[-1])

    xsT = np.zeros((n_cores, D, TOT), BF)
    gixw = np.zeros((n_cores, P, 8 * G), np.int16)
    for c in range(n_cores):
        for g in range(NG):
            u = rows_cg[c][g]
            lut = np.full(n_nodes, -1, np.int32)
            lut[u] = np.arange(len(u), dtype=np.int32)
            xsT[c, :, goff[g]:goff[g] + len(u)] = xsrcT[:, u]
            lo = int(colO[groups[g][0]])
            hi = int(colO[groups[g][-1] + 1])
            loc = lut[gidx[c][:, lo:hi]]
            assert loc.min() >= 0 and loc.max() < 32768
            # wrapped int16 idx layout: idx[j*128+p] at [ (j*128+p)%16,
            # (j*128+p)//16 ] -> [16, 8*nw] blocks, replicated to all 128
            # partitions (one copy per q7 gpsimd core), tile-by-tile
            for t in groups[g]:
                c0, c1 = int(colO[t]), int(colO[t + 1])
                nw = c1 - c0
                L = loc[:, c0 - lo:c1 - lo].T.reshape(-1)
                gixw[c, :, 8 * c0:8 * c1] = np.tile(
                    L.reshape(8 * nw, 16).T, (8, 1))

    return dict(
        S=S, Ws=Ws, NBs=NBs, starts=starts, colO=colO, G=G,
        n_loc_pad=n_loc_pad, gidx=gidx, ew=ew, mask=mask, xdT=xdT,
        node_lists=node_lists, xsT=xsT, Np_used=TOT,
        group_of=group_of, goff=goff, grows=grows, gixw=gixw,
    )


def _b(tile_ap, off, dims):
    """Build a broadcast/strided AP on a tile: partition dim + free dims."""
    import concourse.bass as bass
    return bass.AP(tile_ap.tensor, tile_ap.offset + off,
                   [list(tile_ap.ap[0])] + [list(d) for d in dims])


def _build_program(geos, Nps, n_loc_pads, zero_bias=False, act_prelu=True,
                   loop_reps=1, phase='all'):
    import concourse.mybir as mybir
    import concourse.bacc as bacc
    import concourse.tile as tile
    from concourse.bass import IndirectOffsetOnAxis

    f32 = mybir.dt.float32
    bf16 = mybir.dt.bfloat16
    i32 = mybir.dt.int32
    AL = mybir.AluOpType
    AF = mybir.ActivationFunctionType
    AX = mybir.AxisListType

    nc = bacc.Bacc("TRN2", target_bir_lowering=False, debug=False)

    dram_in = {}

    def inp(name, shape, dt=bf16):
        t = nc.dram_tensor(name, shape, dt, kind="ExternalInput")
        dram_in[name] = t
        return t

    enc_io = []
    for e, sfx in enumerate("st"):
        geo = geos[e]
        nlp = n_loc_pads[e]
        Npe = Nps[e]
        io = dict(
            xsT=inp(f"xsT_{sfx}", [P, Npe]),
            xdT=inp(f"xdT_{sfx}", [P, nlp]),
            gixw=inp(f"gixw_{sfx}", [P, 8 * geo["G"]], mybir.dt.int16),
            ew=inp(f"ew_{sfx}", [P, geo["G"]]),
            mask=inp(f"mask_{sfx}", [P, geo["G"]]),
            wl=inp(f"wl_{sfx}", [P, HC]),
            wr=inp(f"wr_{sfx}", [P, HC]),
            web=inp(f"web_{sfx}", [P, HC]),
            attb=inp(f"attb_{sfx}", [P, HC]),
            bb=inp(f"bb_{sfx}", [P, HC]),
            blb=inp(f"blb_{sfx}", [P, HC]),
            brb=inp(f"brb_{sfx}", [P, HC]),
            out=nc.dram_tensor(f"out_{sfx}", [nlp, HC], f32, kind="ExternalOutput"),
            xl_d=nc.dram_tensor(f"xl_{sfx}", [Npe, HC], bf16, kind="Internal"),
            xr_d=nc.dram_tensor(f"xr_{sfx}", [nlp, HC], bf16, kind="Internal"),
        )
        enc_io.append(io)

    import contextlib
    with tile.TileContext(nc) as tc:
        with (
            tc.tile_pool(name="const", bufs=1) as cpool,
            tc.tile_pool(name="dxin", bufs=3) as dxin,
            tc.tile_pool(name="dpsum", bufs=2, space="PSUM") as dpsum,
            tc.tile_pool(name="dout", bufs=3) as dout,
            tc.tile_pool(name="xlg", bufs=4) as gpool,
            tc.tile_pool(name="zp", bufs=4) as zpool,
            tc.tile_pool(name="xrp", bufs=4) as xrp,
            tc.tile_pool(name="smp", bufs=4) as smp,
            tc.tile_pool(name="outp", bufs=4) as outp,
        ):
            def dense(xT_dram, w_tile, bias_tile, out_dram, nrows, eng_i):
                ntiles = nrows // P
                o = 0
                while o < ntiles:
                    ch = min(CH, ntiles - o)
                    chunk = dxin.tile([P, CH * HC], bf16, tag="dxin")
                    nc.sync.dma_start(
                        out=chunk[:, :ch * HC],
                        in_=xT_dram.ap()[:, o * P:(o + ch) * P])
                    ps = dpsum.tile([P, CH * HC], f32, tag="dpsum")
                    for k in range(ch):
                        nc.tensor.matmul(
                            out=ps[:, k * HC:(k + 1) * HC],
                            lhsT=chunk[:, k * HC:(k + 1) * HC],
                            rhs=w_tile[:], start=True, stop=True)
                    ob = dout.tile([P, CH * HC], bf16, tag="dout")
                    if zero_bias:
                        nc.vector.tensor_scalar_add(
                            out=ob[:, :ch * HC], in0=ps[:, :ch * HC],
                            scalar1=0.0)
                    else:
                        nc.vector.tensor_tensor(
                            out=ob[:, :ch * HC], in0=ps[:, :ch * HC],
                            in1=_b(bias_tile[:], 0, [[0, ch], [1, HC]]),
                            op=AL.add)
                    eng_i += 1
                    dview = out_dram.ap()[o * P:(o + ch) * P, :].rearrange(
                        "(k p) c -> p k c", p=P)
                    nc.sync.dma_start(
                        out=dview, in_=_b(ob[:], 0, [[HC, ch], [1, HC]]))
                    o += ch
                return eng_i

            _ls = contextlib.ExitStack()
            if loop_reps > 1:
                _ls.enter_context(tc.For_i(0, loop_reps, 1))
            NWMAX = max(
                geos[e]["NBs"][i] * geos[e]["Ws"][i]
                for e in range(2) for i in range(geos[e]["S"]))
            enc_ct = [None, None]
            for e in range(2):
                io = enc_io[e]
                geo = geos[e]
                G = geo["G"]
                nlp = n_loc_pads[e]

                ct = {}
                for nm in ("wl", "wr", "web", "attb", "bb", "blb", "brb"):
                    t = cpool.tile([P, HC], bf16, tag=f"{nm}{e}")
                    nc.sync.dma_start(out=t[:], in_=dram_in[f"{nm}_" + "st"[e]].ap())
                    ct[nm] = t
                gix_t = cpool.tile([P, 8 * G], mybir.dt.int16, tag=f"gix{e}")
                nc.sync.dma_start(out=gix_t[:], in_=io["gixw"].ap())
                ew_t = cpool.tile([P, G], bf16, tag=f"ewc{e}")
                nc.sync.dma_start(out=ew_t[:], in_=io["ew"].ap())
                mask_t = cpool.tile([P, G], bf16, tag=f"mk{e}")
                nc.sync.dma_start(out=mask_t[:], in_=io["mask"].ap())
                ct["gix"], ct["ew2"], ct["mk"] = gix_t, ew_t, mask_t

                if not phase.startswith('edge'):
                    ei = dense(io["xsT"], ct["wl"], ct["blb"], io["xl_d"],
                               Nps[e], 0)
                    dense(io["xdT"], ct["wr"], ct["brb"], io["xr_d"], nlp, ei)
                enc_ct[e] = ct

            for e in range(0 if phase == 'dense' else 2):
                io = enc_io[e]
                geo = geos[e]
                S, Ws, NBs, starts, colO = (
                    geo["S"], geo["Ws"], geo["NBs"], geo["starts"], geo["colO"])
                ct = enc_ct[e]
                gix_t, ew_t, mask_t = ct["gix"], ct["ew2"], ct["mk"]

                for t in range(S):
                    W, NB, base = Ws[t], NBs[t], starts[t]
                    cO = int(colO[t])
                    NW = NB * W
                    FW = NW * HC
                    xlg = gpool.tile([P, NWMAX * HC], bf16, tag="xlg")
                    if phase == 'edge_nogather':
                        nc.sync.dma_start(
                            out=xlg[:, :FW],
                            in_=io["xl_d"].ap()[:P * NW, :].rearrange(
                                "(p nw) c -> p nw c", p=P))
                    else:
                        g = int(geo["group_of"][t])
                        go = int(geo["goff"][g])
                        gr = int(geo["grows"][g])
                        # <=7 slots per gather: 896 descs < 1024 SWDGE ring
                        for off in range(0, NW, 7):
                            nwc = min(7, NW - off)
                            nc.gpsimd.dma_gather(
                                out_ap=xlg[:, off * HC:(off + nwc) * HC]
                                .rearrange("p (nw hc) -> p nw hc", nw=nwc),
                                in_ap=io["xl_d"].ap()[go:go + gr, :],
                                idxs_ap=gix_t[:, 8 * (cO + off):
                                              8 * (cO + off + nwc)],
                                num_idxs=P * nwc,
                                num_idxs_reg=P * nwc,
                                elem_size=HC)
                    if phase == 'edge_gonly':
                        o2 = outp.tile([P, NBMAX * HC], f32, tag="o")
                        nc.vector.tensor_scalar_add(
                            out=o2[:, :1], in0=xlg[:, :1], scalar1=1.0)
                        nc.sync.dma_start(
                            out=io["out"].ap()[base:base + 1, :].rearrange(
                                "r c -> r c"),
                            in_=o2[:1, :HC])
                        continue
                    xr2 = xrp.tile([P, NBMAX * HC], bf16, tag="xr")
                    nc.sync.dma_start(
                        out=xr2[:, :NB * HC],
                        in_=io["xr_d"].ap()[base:base + P * NB, :].rearrange(
                            "(p nb) c -> p nb c", p=P))
                    z = zpool.tile([P, NWMAX * HC], bf16, tag="z")
                    # z = ew (x) We   (step-0 last dim -> Pool)
                    nc.gpsimd.tensor_tensor(
                        out=z[:, :FW],
                        in0=_b(ew_t[:], cO, [[1, NW], [0, HC]]),
                        in1=_b(ct["web"][:], 0, [[0, NW], [1, HC]]),
                        op=AL.mult)
                    # z += xr broadcast along w  (packed last -> DVE 2x)
                    nc.vector.tensor_tensor(
                        out=z[:, :FW], in0=z[:, :FW],
                        in1=_b(xr2[:], 0, [[HC, NB], [0, W], [1, HC]]),
                        op=AL.add)
                    # z += xlg  (DVE 2x)
                    nc.vector.tensor_tensor(
                        out=z[:, :FW], in0=z[:, :FW], in1=xlg[:, :FW], op=AL.add)
                    # leaky relu on Act (sim lacks Prelu: DVE stt fallback)
                    if act_prelu:
                        nc.scalar.activation(
                            out=z[:, :FW], in_=z[:, :FW], func=AF.Prelu,
                            alpha=NEG)
                    else:
                        nc.vector.scalar_tensor_tensor(
                            out=z[:, :FW], in0=z[:, :FW], scalar=NEG,
                            in1=z[:, :FW], op0=AL.mult, op1=AL.max)
                    # za = z*att in-place (DVE 2x)
                    nc.vector.tensor_tensor(
                        out=z[:, :FW], in0=z[:, :FW],
                        in1=_b(ct["attb"][:], 0, [[0, NW], [1, HC]]), op=AL.mult)
                    # tree-reduce over C in fp32 partials (logits precision);
                    # (nb, w) merged into nw (stride HC / HC2) for 3D APs
                    CH2 = C // 2
                    HC2 = H * CH2
                    zc = smp.tile([P, NWMAX * HC2], f32, tag="zc")
                    nc.vector.tensor_tensor(
                        out=_b(zc[:], 0, [[HC2, NW], [CH2, H], [1, CH2]]),
                        in0=_b(z[:], 0, [[HC, NW], [C, H], [1, CH2]]),
                        in1=_b(z[:], CH2, [[HC, NW], [C, H], [1, CH2]]),
                        op=AL.add)
                    L = CH2
                    while L > 2:
                        half = L // 2
                        nc.vector.tensor_tensor(
                            out=_b(zc[:], 0, [[HC2, NW], [CH2, H], [1, half]]),
                            in0=_b(zc[:], 0, [[HC2, NW], [CH2, H], [1, half]]),
                            in1=_b(zc[:], half, [[HC2, NW], [CH2, H],
                                                 [1, half]]),
                            op=AL.add)
                        L = half
                    logits = smp.tile([P, H * NWMAX], f32, tag="lg")
                    # traversal (nb, w, h): out logits[nb, h, w]
                    nc.vector.tensor_tensor(
                        out=_b(logits[:], 0, [[H * W, NB], [1, W], [W, H]]),
                        in0=_b(zc[:], 0, [[W * HC2, NB], [HC2, W], [CH2, H]]),
                        in1=_b(zc[:], 1, [[W * HC2, NB], [HC2, W], [CH2, H]]),
                        op=AL.add)
                    lgf = NB * H * W
                    # + mask, then exp (no max subtraction; logits bounded)
                    nc.vector.tensor_tensor(
                        out=logits[:, :lgf], in0=logits[:, :lgf],
                        in1=_b(mask_t[:], cO, [[W, NB], [0, H], [1, W]]),
                        op=AL.add)
                    nc.scalar.activation(
                        out=logits[:, :lgf], in_=logits[:, :lgf], func=AF.Exp)
                    den = smp.tile([P, NBMAX * H], f32, tag="den")
                    nc.vector.tensor_reduce(
                        out=den[:, :NB * H],
                        in_=_b(logits[:], 0, [[H * W, NB], [W, H], [1, W]]),
                        axis=AX.X, op=AL.add)
                    nc.vector.tensor_scalar_add(
                        out=den[:, :NB * H], in0=den[:, :NB * H], scalar1=1e-16)
                    nc.vector.reciprocal(out=den[:, :NB * H], in_=den[:, :NB * H])
                    # alpha = ex * 1/den (fp32, laid out (nb, w, h))
                    alpha = smp.tile([P, H * NWMAX], f32, tag="al")
                    nc.vector.tensor_tensor(
                        out=_b(alpha[:], 0, [[W * H, NB], [1, H], [H, W]]),
                        in0=_b(logits[:], 0, [[H * W, NB], [W, H], [1, W]]),
                        in1=_b(den[:], 0, [[H, NB], [1, H], [0, W]]), op=AL.mult)
                    # msg = xlg * alpha (bcast over c: step-0 last -> Pool)
                    nc.gpsimd.tensor_tensor(
                        out=_b(z[:], 0, [[HC, NW], [C, H], [1, C]]),
                        in0=_b(xlg[:], 0, [[HC, NW], [C, H], [1, C]]),
                        in1=_b(alpha[:], 0, [[H, NW], [1, H], [0, C]]),
                        op=AL.mult)
                    # tree-reduce over W in bf16 (DVE 2x)
                    L = W
                    while L > 1:
                        half = (L + 1) // 2
                        k = L - half
                        nc.vector.tensor_tensor(
                            out=_b(z[:], 0, [[W * HC, NB], [HC, k], [1, HC]]),
                            in0=_b(z[:], 0, [[W * HC, NB], [HC, k], [1, HC]]),
                            in1=_b(z[:], half * HC,
                                   [[W * HC, NB], [HC, k], [1, HC]]),
                            op=AL.add)
                        L = half
                    o2 = outp.tile([P, NBMAX * HC], f32, tag="o")
                    if zero_bias:
                        nc.vector.tensor_scalar_add(
                            out=o2[:, :NB * HC],
                            in0=_b(z[:], 0, [[W * HC, NB], [1, HC]]),
                            scalar1=0.0)
                    else:
                        nc.vector.tensor_tensor(
                            out=o2[:, :NB * HC],
                            in0=_b(z[:], 0, [[W * HC, NB], [1, HC]]),
                            in1=_b(ct["bb"][:], 0, [[0, NB], [1, HC]]),
                            op=AL.add)
                    # ELU = relu(x) + exp(min(x,0)) - 1
                    rt = outp.tile([P, NBMAX * HC], f32, tag="relu")
                    nc.scalar.activation(
                        out=rt[:, :NB * HC], in_=o2[:, :NB * HC], func=AF.Relu)
                    nc.vector.tensor_scalar_min(
                        out=o2[:, :NB * HC], in0=o2[:, :NB * HC], scalar1=0.0)
                    nc.scalar.activation(
                        out=o2[:, :NB * HC], in_=o2[:, :NB * HC], func=AF.Exp)
                    nc.vector.scalar_tensor_tensor(
                        out=o2[:, :NB * HC], in0=o2[:, :NB * HC], scalar=-1.0,
                        in1=rt[:, :NB * HC], op0=AL.add, op1=AL.add)
                    nc.sync.dma_start(
                        out=io["out"].ap()[base:base + P * NB, :].rearrange(
                            "(p nb) c -> p nb c", p=P),
                        in_=_b(o2[:], 0, [[HC, NB], [1, HC]]))
            _ls.close()

    nc.compile()
    return nc


def _elu(x):
    return np.where(x > 0, x, np.expm1(np.minimum(x, 0.0))).astype(np.float32)


def _prep_all(inputs, n_cores):
    s = np.asarray(inputs['s'], np.float32)
    t = np.asarray(inputs['t'], np.float32)
    edges = np.asarray(inputs['edges'])
    ew = np.asarray(inputs['edge_weight'], np.float32)[:, 0]
    src_g, dst_g = edges[0].astype(np.int64), edges[1].astype(np.int64)
    n_nodes = s.shape[0]

    geo_s = _encoder_prep(n_nodes, s, t, src_g, dst_g, ew, n_cores)
    geo_t = _encoder_prep(n_nodes, t, s, dst_g, src_g, ew, n_cores)
    Np = (geo_s["Np_used"], geo_t["Np_used"])

    def bc(v):
        return np.broadcast_to(
            np.asarray(v, np.float32).reshape(-1).astype(BF), (P, HC)).copy()

    consts = {}
    for e, sfx in enumerate("st"):
        consts[f"wl_{sfx}"] = np.asarray(inputs[f"Wl_{sfx}"], np.float32).astype(BF)
        consts[f"wr_{sfx}"] = np.asarray(inputs[f"Wr_{sfx}"], np.float32).astype(BF)
        consts[f"web_{sfx}"] = bc(np.asarray(inputs[f"We_{sfx}"], np.float32)[0])
        consts[f"attb_{sfx}"] = bc(inputs[f"att_{sfx}"])
        consts[f"bb_{sfx}"] = bc(inputs[f"b_{sfx}"])
        consts[f"blb_{sfx}"] = bc(inputs[f"bl_{sfx}"])
        consts[f"brb_{sfx}"] = bc(inputs[f"br_{sfx}"])

    in_maps = []
    for c in range(n_cores):
        m = dict(
            xsT_s=np.ascontiguousarray(geo_s["xsT"][c]),
            xsT_t=np.ascontiguousarray(geo_t["xsT"][c]),
            xdT_s=np.ascontiguousarray(geo_s["xdT"][c]),
            xdT_t=np.ascontiguousarray(geo_t["xdT"][c]),
            gixw_s=geo_s["gixw"][c], gixw_t=geo_t["gixw"][c],
            ew_s=geo_s["ew"][c], ew_t=geo_t["ew"][c],
            mask_s=geo_s["mask"][c], mask_t=geo_t["mask"][c],
        )
        m.update(consts)
        in_maps.append(m)
    return geo_s, geo_t, Np, in_maps


_CACHE = {}


def _get_program(inputs, n_cores=NCORES, act_prelu=True, loop_reps=1,
                 phase='all', **_ignored):
    geo_s, geo_t, Np, in_maps = _prep_all(inputs, n_cores)
    zb = all(
        not np.any(np.asarray(inputs[f"{nm}_{sfx}"]))
        for nm in ("bl", "br", "b") for sfx in "st")
    key = (Np, n_cores, zb, act_prelu, loop_reps, phase,
           tuple(geo_s["Ws"]), tuple(geo_s["NBs"]),
           tuple(geo_t["Ws"]), tuple(geo_t["NBs"]))
    if key not in _CACHE:
        _patch_walrus()
        nc = _build_program(
            [geo_s, geo_t], list(Np), [geo_s["n_loc_pad"], geo_t["n_loc_pad"]],
            zero_bias=zb, act_prelu=act_prelu, loop_reps=loop_reps,
            phase=phase)
        _CACHE[key] = nc
    return _CACHE[key], geo_s, geo_t, in_maps


def _unpermute(inputs, geo_s, geo_t, results, n_cores):
    n_nodes = np.asarray(inputs['s']).shape[0]
    outs = []
    for geo, sfx, bias in (
            (geo_s, "s", inputs["b_s"]), (geo_t, "t", inputs["b_t"])):
        full = np.tile(_elu(np.asarray(bias, np.float32)).reshape(1, HC), (n_nodes, 1))
        for c in range(n_cores):
            nl = geo["node_lists"][c]
            full[nl] = results[c][f"out_{sfx}"][:len(nl)]
        outs.append(full)
    return tuple(outs)


def kernel(**inputs):
    from concourse.bass_interp import get_hw_module
    from concourse import bass_utils
    _patch_walrus()
    nc, geo_s, geo_t, in_maps = _get_program(inputs)
    old_m = nc.m
    nc.m = get_hw_module(nc.m)
    try:
        res = bass_utils.run_bass_kernel_spmd(
            nc, in_maps, core_ids=list(range(NCORES)))
    finally:
        nc.m = old_m
    return _unpermute(inputs, geo_s, geo_t, res.results, NCORES)


# revision 19
# speedup vs baseline: 1.4297x; 1.4297x over previous
"""Trainium2 Bass kernel for nn_DirectedGNNLayer (bipartite GATv2 x2).

Strategy (8 NeuronCores, SPMD — one program, per-core data):
  * Per encoder, partition TARGET (dst) nodes across the 8 cores
    (round-robin by degree rank) so each core owns the full segment
    softmax + aggregation for its nodes — no cross-core reductions.
  * Node-major layout: each supertile holds a block of nodes, NB nodes
    per partition row, each padded to the block's max degree W.
  * bf16 datapath: xl/xr tables, gathers, and all big elementwise ops
    are bf16 (DVE 2x mode needs all-bf16 packed-last-dim operands).
  * One batched indirect DMA per supertile (offset ap [128, NW]) — the
    994ns/instruction SWDGE overhead amortizes over the whole tile.
  * Engine balance: Pool does the two step-0-last-dim broadcasts
    (ee mult, msg mult); Act does prelu+exp; DVE does the packed adds
    and tree reductions (segment sums as log2-depth in-place adds).
  * Softmax max-subtraction is dropped (logits bounded ~|10| on this
    data); exp/den/alpha stay fp32 at the logits level.

kernel(**inputs) takes the FULL problem inputs and returns the FULL
(s_out, t_out) tuple, matching reference.reference().
"""
import sys
import os
import numpy as np
import ml_dtypes

sys.path.insert(0, '/opt/trn_rl_repo')

BF = ml_dtypes.bfloat16

N = 100000
D = 128
E = 800000
H = 4
C = 32
HC = H * C
NEG = 0.2
P = 128
NCORES = 8
CAP = 20      # max NB*W slots per partition row of a supertile
NBMAX = 8
CH = 8        # dense-phase tiles per chunk
NP_PAD = ((N + P - 1) // P) * P


def _patch_walrus():
    from concourse import bass_utils
    if getattr(bass_utils, "_ant_dge_patched", False):
        return
    orig = bass_utils.get_walrus_args

    def patched(*a, **k):
        return orig(*a, **k) + [
            "--dge-levels=io,scalar_dynamic_offset,vector_dynamic_offsets"
        ]

    bass_utils.get_walrus_args = patched
    bass_utils._ant_dge_patched = True


def _encoder_prep(n_nodes, x_src, x_dst, src, dst, edge_w, n_cores):
    """Geometry + per-core host arrays for one encoder.

    src/dst: int arrays [E]; segments (softmax) are over dst.
    Returns a dict; all per-core arrays have identical shapes across cores.
    """
    ne = len(dst)
    deg = np.bincount(dst, minlength=n_nodes)
    order = np.argsort(-deg, kind='stable')
    order = order[deg[order] > 0]
    K = len(order)

    core_of = np.full(n_nodes, -1, np.int32)
    pos_of = np.full(n_nodes, -1, np.int64)
    idx = np.arange(K)
    core_of[order] = (idx % n_cores).astype(np.int32)
    pos_of[order] = idx // n_cores
    n_loc = (K + n_cores - 1) // n_cores

    deg_rank = deg[order[0::n_cores]]

    Ws, NBs, starts = [], [], []
    pos = 0
    while pos < n_loc:
        W = int(deg_rank[pos]) if pos < len(deg_rank) else 1
        W = max(W, 1)
        NB = max(1, min(NBMAX, CAP // W))
        starts.append(pos)
        Ws.append(W)
        NBs.append(NB)
        pos += P * NB
    n_loc_pad = pos
    S = len(Ws)
    colO = np.zeros(S + 1, np.int64)
    for t in range(S):
        colO[t + 1] = colO[t] + NBs[t] * Ws[t]
    G = int(colO[-1])

    row_of = np.empty(n_loc_pad, np.int64)
    colb_of = np.empty(n_loc_pad, np.int64)
    for t in range(S):
        r = np.arange(P * NBs[t])
        sl = slice(starts[t], starts[t] + P * NBs[t])
        row_of[sl] = r // NBs[t]
        colb_of[sl] = colO[t] + (r % NBs[t]) * Ws[t]

    sidx = np.argsort(dst, kind='stable')
    sdst = dst[sidx]
    first = np.r_[True, sdst[1:] != sdst[:-1]]
    run_starts_pos = np.flatnonzero(first)
    run_id = np.cumsum(first) - 1
    w_sorted = np.arange(ne) - run_starts_pos[run_id]
    w_of = np.empty(ne, np.int64)
    w_of[sidx] = w_sorted

    c_e = core_of[dst]
    j_e = pos_of[dst]
    row_e = row_of[j_e]
    col_e = colb_of[j_e] + w_of

    gidx = np.zeros((n_cores, P, G), np.int32)
    ew = np.zeros((n_cores, P, G), BF)
    mask = np.full((n_cores, P, G), -1e30, BF)
    gidx[c_e, row_e, col_e] = src.astype(np.int32)
    ew[c_e, row_e, col_e] = edge_w.astype(BF)
    mask[c_e, row_e, col_e] = 0.0

    # host-packed local dst features, transposed for matmul lhsT (bf16)
    xdT = np.zeros((n_cores, D, n_loc_pad), BF)
    node_lists = []
    xdstT = np.ascontiguousarray(x_dst.T).astype(BF)
    for c in range(n_cores):
        nl = order[c::n_cores]
        node_lists.append(nl)
        xdT[c, :, :len(nl)] = xdstT[:, nl]

    # Group supertiles so each group's distinct sources stay < 32768 on
    # EVERY core (dma_gather idx is int16).  Each group gets its own
    # sub-table slice of xsT; gidx values are remapped group-locally.
    GCAP = 31500
    xsrcT = np.ascontiguousarray(x_src.T).astype(BF)
    seen = np.zeros((n_cores, n_nodes), bool)
    gcount = np.zeros(n_cores, np.int64)
    group_of = np.zeros(S, np.int64)
    groups = [[]]
    for t in range(S):
        sl = slice(int(colO[t]), int(colO[t + 1]))
        news = [np.unique(gidx[c][:, sl]) for c in range(n_cores)]
        adds = [np.count_nonzero(~seen[c][news[c]]) for c in range(n_cores)]
        if any(gcount[c] + adds[c] > GCAP for c in range(n_cores)):
            groups.append([])
            seen[:] = False
            gcount[:] = 0
        g = len(groups) - 1
        groups[g].append(t)
        group_of[t] = g
        for c in range(n_cores):
            gcount[c] += np.count_nonzero(~seen[c][news[c]])
            seen[c][news[c]] = True

    # per-group sub-tables, padded to a common (max-over-core) size
    NG = len(groups)
    grows = np.zeros(NG, np.int64)
    rows_cg = [[None] * NG for _ in range(n_cores)]
    for g in range(NG):
        lo = int(colO[groups[g][0]])
        hi = int(colO[groups[g][-1] + 1])
        for c in range(n_cores):
            u = np.unique(gidx[c][:, lo:hi])
            rows_cg[c][g] = u
            grows[g] = max(grows[g], len(u))
    grows = ((grows + P - 1) // P) * P
    goff = np.zeros(NG + 1, np.int64)
    for g in range(NG):
        goff[g + 1] = goff[g] + grows[g]
    TOT = int(goff[-1])

    xsT = np.zeros((n_cores, D, TOT), BF)
    gixw = np.zeros((n_cores, P, 8 * G), np.int16)
    for c in range(n_cores):
        for g in range(NG):
            u = rows_cg[c][g]
            lut = np.full(n_nodes, -1, np.int32)
            lut[u] = np.arange(len(u), dtype=np.int32)
            xsT[c, :, goff[g]:goff[g] + len(u)] = xsrcT[:, u]
            lo = int(colO[groups[g][0]])
            hi = int(colO[groups[g][-1] + 1])
            loc = lut[gidx[c][:, lo:hi]]
            assert loc.min() >= 0 and loc.max() < 32768
            # wrapped int16 idx layout: idx[j*128+p] at [ (j*128+p)%16,
            # (j*128+p)//16 ] -> [16, 8*nw] blocks, replicated to all 128
            # partitions (one copy per q7 gpsimd core), tile-by-tile
            for t in groups[g]:
                c0, c1 = int(colO[t]), int(colO[t + 1])
                nw = c1 - c0
                L = loc[:, c0 - lo:c1 - lo].T.reshape(-1)
                gixw[c, :, 8 * c0:8 * c1] = np.tile(
                    L.reshape(8 * nw, 16).T, (8, 1))

    return dict(
        S=S, Ws=Ws, NBs=NBs, starts=starts, colO=colO, G=G,
        n_loc_pad=n_loc_pad, gidx=gidx, ew=ew, mask=mask, xdT=xdT,
        node_lists=node_lists, xsT=xsT, Np_used=TOT,
        group_of=group_of, goff=goff, grows=grows, gixw=gixw,
    )


def _b(tile_ap, off, dims):
    """Build a broadcast/strided AP on a tile: partition dim + free dims."""
    import concourse.bass as bass
    return bass.AP(tile_ap.tensor, tile_ap.offset + off,
                   [list(tile_ap.ap[0])] + [list(d) for d in dims])


def _build_program(geos, Nps, n_loc_pads, zero_bias=False, act_prelu=True,
                   loop_reps=1, phase='all'):
    import concourse.mybir as mybir
    import concourse.bacc as bacc
    import concourse.tile as tile
    from concourse.bass import IndirectOffsetOnAxis

    f32 = mybir.dt.float32
    bf16 = mybir.dt.bfloat16
    i32 = mybir.dt.int32
    AL = mybir.AluOpType
    AF = mybir.ActivationFunctionType
    AX = mybir.AxisListType

    nc = bacc.Bacc("TRN2", target_bir_lowering=False, debug=False,
                   num_swdge_queues=4)

    dram_in = {}
    qrr = [0]

    def inp(name, shape, dt=bf16):
        t = nc.dram_tensor(name, shape, dt, kind="ExternalInput")
        dram_in[name] = t
        return t

    enc_io = []
    for e, sfx in enumerate("st"):
        geo = geos[e]
        nlp = n_loc_pads[e]
        Npe = Nps[e]
        io = dict(
            xsT=inp(f"xsT_{sfx}", [P, Npe]),
            xdT=inp(f"xdT_{sfx}", [P, nlp]),
            gixw=inp(f"gixw_{sfx}", [P, 8 * geo["G"]], mybir.dt.int16),
            ew=inp(f"ew_{sfx}", [P, geo["G"]]),
            mask=inp(f"mask_{sfx}", [P, geo["G"]]),
            wl=inp(f"wl_{sfx}", [P, HC]),
            wr=inp(f"wr_{sfx}", [P, HC]),
            web=inp(f"web_{sfx}", [P, HC]),
            attb=inp(f"attb_{sfx}", [P, HC]),
            bb=inp(f"bb_{sfx}", [P, HC]),
            blb=inp(f"blb_{sfx}", [P, HC]),
            brb=inp(f"brb_{sfx}", [P, HC]),
            out=nc.dram_tensor(f"out_{sfx}", [nlp, HC], f32, kind="ExternalOutput"),
            xl_d=nc.dram_tensor(f"xl_{sfx}", [Npe, HC], bf16, kind="Internal"),
            xr_d=nc.dram_tensor(f"xr_{sfx}", [nlp, HC], bf16, kind="Internal"),
        )
        enc_io.append(io)

    import contextlib
    with tile.TileContext(nc) as tc:
        with (
            tc.tile_pool(name="const", bufs=1) as cpool,
            tc.tile_pool(name="dxin", bufs=3) as dxin,
            tc.tile_pool(name="dpsum", bufs=2, space="PSUM") as dpsum,
            tc.tile_pool(name="dout", bufs=3) as dout,
            tc.tile_pool(name="xlg", bufs=4) as gpool,
            tc.tile_pool(name="zp", bufs=4) as zpool,
            tc.tile_pool(name="xrp", bufs=4) as xrp,
            tc.tile_pool(name="smp", bufs=4) as smp,
            tc.tile_pool(name="outp", bufs=4) as outp,
        ):
            def dense(xT_dram, w_tile, bias_tile, out_dram, nrows, eng_i):
                ntiles = nrows // P
                o = 0
                while o < ntiles:
                    ch = min(CH, ntiles - o)
                    chunk = dxin.tile([P, CH * HC], bf16, tag="dxin")
                    nc.sync.dma_start(
                        out=chunk[:, :ch * HC],
                        in_=xT_dram.ap()[:, o * P:(o + ch) * P])
                    ps = dpsum.tile([P, CH * HC], f32, tag="dpsum")
                    for k in range(ch):
                        nc.tensor.matmul(
                            out=ps[:, k * HC:(k + 1) * HC],
                            lhsT=chunk[:, k * HC:(k + 1) * HC],
                            rhs=w_tile[:], start=True, stop=True)
                    ob = dout.tile([P, CH * HC], bf16, tag="dout")
                    if zero_bias:
                        nc.vector.tensor_scalar_add(
                            out=ob[:, :ch * HC], in0=ps[:, :ch * HC],
                            scalar1=0.0)
                    else:
                        nc.vector.tensor_tensor(
                            out=ob[:, :ch * HC], in0=ps[:, :ch * HC],
                            in1=_b(bias_tile[:], 0, [[0, ch], [1, HC]]),
                            op=AL.add)
                    eng_i += 1
                    dview = out_dram.ap()[o * P:(o + ch) * P, :].rearrange(
                        "(k p) c -> p k c", p=P)
                    nc.sync.dma_start(
                        out=dview, in_=_b(ob[:], 0, [[HC, ch], [1, HC]]))
                    o += ch
                return eng_i

            _ls = contextlib.ExitStack()
            if loop_reps > 1:
                _ls.enter_context(tc.For_i(0, loop_reps, 1))
            NWMAX = max(
                geos[e]["NBs"][i] * geos[e]["Ws"][i]
                for e in range(2) for i in range(geos[e]["S"]))
            enc_ct = [None, None]
            for e in range(2):
                io = enc_io[e]
                geo = geos[e]
                G = geo["G"]
                nlp = n_loc_pads[e]

                ct = {}
                for nm in ("wl", "wr", "web", "attb", "bb", "blb", "brb"):
                    t = cpool.tile([P, HC], bf16, tag=f"{nm}{e}")
                    nc.sync.dma_start(out=t[:], in_=dram_in[f"{nm}_" + "st"[e]].ap())
                    ct[nm] = t
                gix_t = cpool.tile([P, 8 * G], mybir.dt.int16, tag=f"gix{e}")
                nc.sync.dma_start(out=gix_t[:], in_=io["gixw"].ap())
                ew_t = cpool.tile([P, G], bf16, tag=f"ewc{e}")
                nc.sync.dma_start(out=ew_t[:], in_=io["ew"].ap())
                mask_t = cpool.tile([P, G], bf16, tag=f"mk{e}")
                nc.sync.dma_start(out=mask_t[:], in_=io["mask"].ap())
                ct["gix"], ct["ew2"], ct["mk"] = gix_t, ew_t, mask_t

                if not phase.startswith('edge'):
                    ei = dense(io["xsT"], ct["wl"], ct["blb"], io["xl_d"],
                               Nps[e], 0)
                    dense(io["xdT"], ct["wr"], ct["brb"], io["xr_d"], nlp, ei)
                enc_ct[e] = ct

            for e in range(0 if phase == 'dense' else 2):
                io = enc_io[e]
                geo = geos[e]
                S, Ws, NBs, starts, colO = (
                    geo["S"], geo["Ws"], geo["NBs"], geo["starts"], geo["colO"])
                ct = enc_ct[e]
                gix_t, ew_t, mask_t = ct["gix"], ct["ew2"], ct["mk"]

                for t in range(S):
                    W, NB, base = Ws[t], NBs[t], starts[t]
                    cO = int(colO[t])
                    NW = NB * W
                    FW = NW * HC
                    xlg = gpool.tile([P, NWMAX * HC], bf16, tag="xlg")
                    if phase == 'edge_nogather':
                        nc.sync.dma_start(
                            out=xlg[:, :FW],
                            in_=io["xl_d"].ap()[:P * NW, :].rearrange(
                                "(p nw) c -> p nw c", p=P))
                    else:
                        g = int(geo["group_of"][t])
                        go = int(geo["goff"][g])
                        gr = int(geo["grows"][g])
                        # <=7 slots per gather: 896 descs < 1024 SWDGE ring;
                        # round-robin the 4 SWDGE queues for DMA parallelism
                        for off in range(0, NW, 7):
                            nwc = min(7, NW - off)
                            nc.gpsimd.dma_gather(
                                out_ap=xlg[:, off * HC:(off + nwc) * HC]
                                .rearrange("p (nw hc) -> p nw hc", nw=nwc),
                                in_ap=io["xl_d"].ap()[go:go + gr, :],
                                idxs_ap=gix_t[:, 8 * (cO + off):
                                              8 * (cO + off + nwc)],
                                num_idxs=P * nwc,
                                num_idxs_reg=P * nwc,
                                elem_size=HC,
                                queue_num=qrr[0] % 4)
                            qrr[0] += 1
                    if phase == 'edge_gonly':
                        o2 = outp.tile([P, NBMAX * HC], f32, tag="o")
                        nc.vector.tensor_scalar_add(
                            out=o2[:, :1], in0=xlg[:, :1], scalar1=1.0)
                        nc.sync.dma_start(
                            out=io["out"].ap()[base:base + 1, :].rearrange(
                                "r c -> r c"),
                            in_=o2[:1, :HC])
                        continue
                    xr2 = xrp.tile([P, NBMAX * HC], bf16, tag="xr")
                    nc.sync.dma_start(
                        out=xr2[:, :NB * HC],
                        in_=io["xr_d"].ap()[base:base + P * NB, :].rearrange(
                            "(p nb) c -> p nb c", p=P))
                    z = zpool.tile([P, NWMAX * HC], bf16, tag="z")
                    # z = ew (x) We   (step-0 last dim -> Pool)
                    nc.gpsimd.tensor_tensor(
                        out=z[:, :FW],
                        in0=_b(ew_t[:], cO, [[1, NW], [0, HC]]),
                        in1=_b(ct["web"][:], 0, [[0, NW], [1, HC]]),
                        op=AL.mult)
                    # z += xr broadcast along w  (packed last -> DVE 2x)
                    nc.vector.tensor_tensor(
                        out=z[:, :FW], in0=z[:, :FW],
                        in1=_b(xr2[:], 0, [[HC, NB], [0, W], [1, HC]]),
                        op=AL.add)
                    # z += xlg  (DVE 2x)
                    nc.vector.tensor_tensor(
                        out=z[:, :FW], in0=z[:, :FW], in1=xlg[:, :FW], op=AL.add)
                    # leaky relu on Act (sim lacks Prelu: DVE stt fallback)
                    if act_prelu:
                        nc.scalar.activation(
                            out=z[:, :FW], in_=z[:, :FW], func=AF.Prelu,
                            alpha=NEG)
                    else:
                        nc.vector.scalar_tensor_tensor(
                            out=z[:, :FW], in0=z[:, :FW], scalar=NEG,
                            in1=z[:, :FW], op0=AL.mult, op1=AL.max)
                    # za = z*att in-place (DVE 2x)
                    nc.vector.tensor_tensor(
                        out=z[:, :FW], in0=z[:, :FW],
                        in1=_b(ct["attb"][:], 0, [[0, NW], [1, HC]]), op=AL.mult)
                    # tree-reduce over C in fp32 partials (logits precision);
                    # (nb, w) merged into nw (stride HC / HC2) for 3D APs
                    CH2 = C // 2
                    HC2 = H * CH2
                    zc = smp.tile([P, NWMAX * HC2], f32, tag="zc")
                    nc.vector.tensor_tensor(
                        out=_b(zc[:], 0, [[HC2, NW], [CH2, H], [1, CH2]]),
                        in0=_b(z[:], 0, [[HC, NW], [C, H], [1, CH2]]),
                        in1=_b(z[:], CH2, [[HC, NW], [C, H], [1, CH2]]),
                        op=AL.add)
                    L = CH2
                    while L > 2:
                        half = L // 2
                        nc.vector.tensor_tensor(
                            out=_b(zc[:], 0, [[HC2, NW], [CH2, H], [1, half]]),
                            in0=_b(zc[:], 0, [[HC2, NW], [CH2, H], [1, half]]),
                            in1=_b(zc[:], half, [[HC2, NW], [CH2, H],
                                                 [1, half]]),
                            op=AL.add)
                        L = half
                    logits = smp.tile([P, H * NWMAX], f32, tag="lg")
                    # traversal (nb, w, h): out logits[nb, h, w]
                    nc.vector.tensor_tensor(
                        out=_b(logits[:], 0, [[H * W, NB], [1, W], [W, H]]),
                        in0=_b(zc[:], 0, [[W * HC2, NB], [HC2, W], [CH2, H]]),
                        in1=_b(zc[:], 1, [[W * HC2, NB], [HC2, W], [CH2, H]]),
                        op=AL.add)
                    lgf = NB * H * W
                    # + mask, then exp (no max subtraction; logits bounded)
                    nc.vector.tensor_tensor(
                        out=logits[:, :lgf], in0=logits[:, :lgf],
                        in1=_b(mask_t[:], cO, [[W, NB], [0, H], [1, W]]),
                        op=AL.add)
                    nc.scalar.activation(
                        out=logits[:, :lgf], in_=logits[:, :lgf], func=AF.Exp)
                    den = smp.tile([P, NBMAX * H], f32, tag="den")
                    nc.vector.tensor_reduce(
                        out=den[:, :NB * H],
                        in_=_b(logits[:], 0, [[H * W, NB], [W, H], [1, W]]),
                        axis=AX.X, op=AL.add)
                    nc.vector.tensor_scalar_add(
                        out=den[:, :NB * H], in0=den[:, :NB * H], scalar1=1e-16)
                    nc.vector.reciprocal(out=den[:, :NB * H], in_=den[:, :NB * H])
                    # alpha = ex * 1/den (fp32, laid out (nb, w, h))
                    alpha = smp.tile([P, H * NWMAX], f32, tag="al")
                    nc.vector.tensor_tensor(
                        out=_b(alpha[:], 0, [[W * H, NB], [1, H], [H, W]]),
                        in0=_b(logits[:], 0, [[H * W, NB], [W, H], [1, W]]),
                        in1=_b(den[:], 0, [[H, NB], [1, H], [0, W]]), op=AL.mult)
                    # msg = xlg * alpha (bcast over c: step-0 last -> Pool)
                    nc.gpsimd.tensor_tensor(
                        out=_b(z[:], 0, [[HC, NW], [C, H], [1, C]]),
                        in0=_b(xlg[:], 0, [[HC, NW], [C, H], [1, C]]),
                        in1=_b(alpha[:], 0, [[H, NW], [1, H], [0, C]]),
                        op=AL.mult)
                    # tree-reduce over W in bf16 (DVE 2x)
                    L = W
                    while L > 1:
                        half = (L + 1) // 2
                        k = L - half
                        nc.vector.tensor_tensor(
                            out=_b(z[:], 0, [[W * HC, NB], [HC, k], [1, HC]]),
                            in0=_b(z[:], 0, [[W * HC, NB], [HC, k], [1, HC]]),
                            in1=_b(z[:], half * HC,
                                   [[W * HC, NB], [HC, k], [1, HC]]),
                            op=AL.add)
                        L = half
                    o2 = outp.tile([P, NBMAX * HC], f32, tag="o")
                    if zero_bias:
                        nc.vector.tensor_scalar_add(
                            out=o2[:, :NB * HC],
                            in0=_b(z[:], 0, [[W * HC, NB], [1, HC]]),
                            scalar1=0.0)
                    else:
                        nc.vector.tensor_tensor(
                            out=o2[:, :NB * HC],
                            in0=_b(z[:], 0, [[W * HC, NB], [1, HC]]),
                            in1=_b(ct["bb"][:], 0, [[0, NB], [1, HC]]),
                            op=AL.add)
                    # ELU = relu(x) + exp(min(x,0)) - 1
                    rt = outp.tile([P, NBMAX * HC], f32, tag="relu")
                    nc.scalar.activation(
                        out=rt[:, :NB * HC], in_=o2[:, :NB * HC], func=AF.Relu)
                    nc.vector.tensor_scalar_min(
                        out=o2[:, :NB * HC], in0=o2[:, :NB * HC], scalar1=0.0)
                    nc.scalar.activation(
                        out=o2[:, :NB * HC], in_=o2[:, :NB * HC], func=AF.Exp)
                    nc.vector.scalar_tensor_tensor(
                        out=o2[:, :NB * HC], in0=o2[:, :NB * HC], scalar=-1.0,
                        in1=rt[:, :NB * HC], op0=AL.add, op1=AL.add)
                    nc.sync.dma_start(
                        out=io["out"].ap()[base:base + P * NB, :].rearrange(
                            "(p nb) c -> p nb c", p=P),
                        in_=_b(o2[:], 0, [[HC, NB], [1, HC]]))
            _ls.close()

    nc.compile()
    return nc


def _elu(x):
    return np.where(x > 0, x, np.expm1(np.minimum(x, 0.0))).astype(np.float32)


def _prep_all(inputs, n_cores):
    s = np.asarray(inputs['s'], np.float32)
    t = np.asarray(inputs['t'], np.float32)
    edges = np.asarray(inputs['edges'])
    ew = np.asarray(inputs['edge_weight'], np.float32)[:, 0]
    src_g, dst_g = edges[0].astype(np.int64), edges[1].astype(np.int64)
    n_nodes = s.shape[0]

    geo_s = _encoder_prep(n_nodes, s, t, src_g, dst_g, ew, n_cores)
    geo_t = _encoder_prep(n_nodes, t, s, dst_g, src_g, ew, n_cores)
    Np = (geo_s["Np_used"], geo_t["Np_used"])

    def bc(v):
        return np.broadcast_to(
            np.asarray(v, np.float32).reshape(-1).astype(BF), (P, HC)).copy()

    consts = {}
    for e, sfx in enumerate("st"):
        consts[f"wl_{sfx}"] = np.asarray(inputs[f"Wl_{sfx}"], np.float32).astype(BF)
        consts[f"wr_{sfx}"] = np.asarray(inputs[f"Wr_{sfx}"], np.float32).astype(BF)
        consts[f"web_{sfx}"] = bc(np.asarray(inputs[f"We_{sfx}"], np.float32)[0])
        consts[f"attb_{sfx}"] = bc(inputs[f"att_{sfx}"])
        consts[f"bb_{sfx}"] = bc(inputs[f"b_{sfx}"])
        consts[f"blb_{sfx}"] = bc(inputs[f"bl_{sfx}"])
        consts[f"brb_{sfx}"] = bc(inputs[f"br_{sfx}"])

    in_maps = []
    for c in range(n_cores):
        m = dict(
            xsT_s=np.ascontiguousarray(geo_s["xsT"][c]),
            xsT_t=np.ascontiguousarray(geo_t["xsT"][c]),
            xdT_s=np.ascontiguousarray(geo_s["xdT"][c]),
            xdT_t=np.ascontiguousarray(geo_t["xdT"][c]),
            gixw_s=geo_s["gixw"][c], gixw_t=geo_t["gixw"][c],
            ew_s=geo_s["ew"][c], ew_t=geo_t["ew"][c],
            mask_s=geo_s["mask"][c], mask_t=geo_t["mask"][c],
        )
        m.update(consts)
        in_maps.append(m)
    return geo_s, geo_t, Np, in_maps


_CACHE = {}


def _get_program(inputs, n_cores=NCORES, act_prelu=True, loop_reps=1,
                 phase='all', **_ignored):
    geo_s, geo_t, Np, in_maps = _prep_all(inputs, n_cores)
    zb = all(
        not np.any(np.asarray(inputs[f"{nm}_{sfx}"]))
        for nm in ("bl", "br", "b") for sfx in "st")
    key = (Np, n_cores, zb, act_prelu, loop_reps, phase,
           tuple(geo_s["Ws"]), tuple(geo_s["NBs"]),
           tuple(geo_t["Ws"]), tuple(geo_t["NBs"]))
    if key not in _CACHE:
        _patch_walrus()
        nc = _build_program(
            [geo_s, geo_t], list(Np), [geo_s["n_loc_pad"], geo_t["n_loc_pad"]],
            zero_bias=zb, act_prelu=act_prelu, loop_reps=loop_reps,
            phase=phase)
        _CACHE[key] = nc
    return _CACHE[key], geo_s, geo_t, in_maps


def _unpermute(inputs, geo_s, geo_t, results, n_cores):
    n_nodes = np.asarray(inputs['s']).shape[0]
    outs = []
    for geo, sfx, bias in (
            (geo_s, "s", inputs["b_s"]), (geo_t, "t", inputs["b_t"])):
        full = np.tile(_elu(np.asarray(bias, np.float32)).reshape(1, HC), (n_nodes, 1))
        for c in range(n_cores):
            nl = geo["node_lists"][c]
            full[nl] = results[c][f"out_{sfx}"][:len(nl)]
        outs.append(full)
    return tuple(outs)


def kernel(**inputs):
    from concourse.bass_interp import get_hw_module
    from concourse import bass_utils
    _patch_walrus()
    nc, geo_s, geo_t, in_maps = _get_program(inputs)
    old_m = nc.m
    nc.m = get_hw_module(nc.m)
    try:
        res = bass_utils.run_bass_kernel_spmd(
            nc, in_maps, core_ids=list(range(NCORES)))
    finally:
        nc.m = old_m
    return _unpermute(inputs, geo_s, geo_t, res.results, NCORES)
